# revision 14
# baseline (speedup 1.0000x reference)
"""Trainium2 Bass kernel for nn_MergeModel (GNN message passing + MLP chain).

Strategy (8 NeuronCores, SPMD, no collectives):
  - Nodes are partitioned into 8 contiguous ranges of 2500 (one per core).
  - Edges are partitioned by destination node (edge_index[1]), so each
    core's scatter_mean is fully local.  Host-side prep buckets edges by
    128-node destination group and pads each group to a fixed number of
    128-edge chunks (SPMD requires uniform shapes).
  - Scatter-sum runs on the TensorEngine: for each 128-edge chunk a
    one-hot matrix (built on-device with iota + is_equal) is matmul'd
    against [edge_attr | 1] to accumulate per-node sums and counts in
    PSUM.  Division by max(count,1) gives the mean; a PE transpose puts
    the result in feature-major layout.
  - The 4-GEMM MLP chain runs feature-major (activations stored
    transposed, [features, rows]) so no transposes are needed between
    layers.  LayerNorm partition reductions are done with ones-vector
    matmuls on the PE; mean/rstd rows are broadcast across partitions
    with a GpSimd partition_broadcast.
  - All matmuls use float32r (TF32-like, 1 cycle/row for N>=256; ~16x
    more accurate than bf16).
  - Output is written feature-major [512, rows] and transposed on the
    host during unsharding.
"""

import numpy as np

import concourse.bass as bass
import concourse.tile as tile
from concourse import bacc, mybir
from concourse.bass_utils import run_bass_kernel_spmd
from concourse import masks

F32 = mybir.dt.float32
F32R = mybir.dt.float32r
AF = mybir.ActivationFunctionType
ALU = mybir.AluOpType

# ---------------------------------------------------------------- config

FULL_CFG = dict(
    n_cores=8,
    nodes_pc=2500,     # real nodes per core
    groups=20,         # 128-node groups per core (20*128 = 2560 slots)
    ch_g=36,           # 128-edge chunks per group (36*128 = 4608 edge slots)
    f=256,             # node/edge feature dim
    d=2048,            # hidden width
    d_out=512,         # final out dim
    r_blk=512,         # rows per GEMM block
    eps=1e-5,
    edge_batch=4,      # chunks per edge DMA
)


def _derived(cfg):
    rows = cfg["groups"] * 128
    chunks = cfg["groups"] * cfg["ch_g"]
    n_blks = rows // cfg["r_blk"]
    assert rows % cfg["r_blk"] == 0
    assert cfg["ch_g"] % cfg["edge_batch"] == 0
    return rows, chunks, n_blks


# ---------------------------------------------------------- device program

def build_program(cfg):
    rows, chunks, n_blks = _derived(cfg)
    F = cfg["f"]
    D = cfg["d"]
    DO = cfg["d_out"]
    R = cfg["r_blk"]
    G = cfg["groups"]
    CHG = cfg["ch_g"]
    EB = cfg["edge_batch"]
    FE = F + 2                      # edge features + ones col + pad (fp32r needs even N)
    MT = D // 128                   # m-tiles of hidden dim
    MO = DO // 128                  # m-tiles of out dim
    K1 = (2 * F) // 128             # k-tiles of h0 (agg + x)
    K3 = MT + K1                    # k-tiles of GEMM3 input (h5 | h0)
    QW = 512                        # weight-stream column width
    Q = D // QW                     # q-passes over hidden out dim
    GPB = R // 128                  # groups per block

    nc = bacc.Bacc("TRN2", target_bir_lowering=False, debug=False,
                   num_devices=cfg["n_cores"])

    edges_t = nc.dram_tensor("edges_t", [128, chunks * FE], F32,
                             kind="ExternalInput").ap()
    cols_t = nc.dram_tensor("cols_t", [128, chunks], F32,
                            kind="ExternalInput").ap()
    xT = nc.dram_tensor("xT", [K1 // 2, 128, rows], F32,
                        kind="ExternalInput").ap()
    W1t = nc.dram_tensor("W1t", [K1, Q, 128, QW], F32R,
                         kind="ExternalInput").ap()
    W3t = nc.dram_tensor("W3t", [MT, Q, 128, QW], F32R,
                         kind="ExternalInput").ap()
    W6t = nc.dram_tensor("W6t", [K3, Q, 128, QW], F32R,
                         kind="ExternalInput").ap()
    W8t = nc.dram_tensor("W8t", [MT, 1, 128, DO], F32R,
                         kind="ExternalInput").ap()
    b1t = nc.dram_tensor("b1t", [128, MT], F32, kind="ExternalInput").ap()
    b3t = nc.dram_tensor("b3t", [128, MT], F32, kind="ExternalInput").ap()
    b6t = nc.dram_tensor("b6t", [128, MT], F32, kind="ExternalInput").ap()
    b8t = nc.dram_tensor("b8t", [128, MO], F32, kind="ExternalInput").ap()
    gnt = nc.dram_tensor("gnt", [128, MT], F32, kind="ExternalInput").ap()
    bnt = nc.dram_tensor("bnt", [128, MT], F32, kind="ExternalInput").ap()
    outT = nc.dram_tensor("outT", [DO, rows], F32, kind="ExternalOutput").ap()

    with tile.TileContext(nc) as tc:
        with (
            tc.tile_pool(name="const", bufs=1) as p_const,
            tc.tile_pool(name="h0", bufs=n_blks) as p_h0,
            tc.tile_pool(name="h2h5", bufs=MT + 4) as p_h2,
            tc.tile_pool(name="h4h7", bufs=MT + 4) as p_h4,
            tc.tile_pool(name="estage", bufs=3) as p_estage,
            tc.tile_pool(name="econv", bufs=3) as p_econv,
            tc.tile_pool(name="oh", bufs=8) as p_oh,
            tc.tile_pool(name="wconv", bufs=6) as p_wconv,
            tc.tile_pool(name="small", bufs=2) as p_small,
            tc.tile_pool(name="lnrow", bufs=1) as p_lnrow,
            tc.tile_pool(name="lnb", bufs=1) as p_lnb,
            tc.tile_pool(name="sq", bufs=3) as p_sq,
            tc.tile_pool(name="outsb", bufs=2) as p_outsb,
            tc.tile_pool(name="ps_sc", bufs=2, space="PSUM") as ps_sc,
            tc.tile_pool(name="ps_gemm", bufs=1, space="PSUM") as ps_gemm,
            tc.tile_pool(name="ps_pst", bufs=1, space="PSUM") as ps_pst,
            tc.tile_pool(name="ps_ln", bufs=1, space="PSUM") as ps_ln,
        ):
            # ---------------- constants
            iota_i = p_const.tile([128, 128], mybir.dt.int32)
            nc.gpsimd.iota(iota_i[:], pattern=[[1, 128]], base=0,
                           channel_multiplier=0)
            iota_f = p_const.tile([128, 128], F32)
            nc.vector.tensor_copy(iota_f[:], iota_i[:])
            ident = p_const.tile([128, 128], F32)
            masks.make_identity(nc, ident[:])
            ones_f = p_const.tile([128, 1], F32)
            nc.gpsimd.memset(ones_f[:], 1.0)
            ones_r = p_const.tile([128, 1], F32R)
            nc.vector.tensor_copy(ones_r[:], ones_f[:])
            eps_t = p_const.tile([1, 1], F32)
            nc.gpsimd.memset(eps_t[:], cfg["eps"])

            cols_sb = p_const.tile([128, chunks], F32)
            nc.sync.dma_start(cols_sb[:], cols_t)
            b1s = p_const.tile([128, MT], F32)
            nc.sync.dma_start(b1s[:], b1t)
            b3s = p_const.tile([128, MT], F32)
            nc.sync.dma_start(b3s[:], b3t)
            b6s = p_const.tile([128, MT], F32)
            nc.sync.dma_start(b6s[:], b6t)
            b8s = p_const.tile([128, MO], F32)
            nc.sync.dma_start(b8s[:], b8t)
            gns = p_const.tile([128, MT], F32)
            nc.sync.dma_start(gns[:], gnt)
            bns = p_const.tile([128, MT], F32)
            nc.sync.dma_start(bns[:], bnt)

            h0_tiles = [[None] * K1 for _ in range(n_blks)]

            # ---------------- phase emitters
            def emit_scatter_block(b):
                """Scatter-mean for groups [b*GPB, (b+1)*GPB) -> h0 tiles."""
                for half in range(K1 // 2):
                    t = p_h0.tile([128, R], F32R, tag=f"h0a{half}")
                    h0_tiles[b][half] = t
                for gi in range(GPB):
                    g = b * GPB + gi
                    ps = ps_sc.tile([128, FE], F32)
                    for j0 in range(0, CHG, EB):
                        est = p_estage.tile([128, EB * FE], F32)
                        ch0 = g * CHG + j0
                        nc.sync.dma_start(
                            est[:], edges_t[:, ch0 * FE:(ch0 + EB) * FE])
                        ecv = p_econv.tile([128, EB * FE], F32R)
                        nc.gpsimd.tensor_copy(ecv[:], est[:])
                        for jj in range(EB):
                            ch = ch0 + jj
                            oh = p_oh.tile([128, 128], F32R)
                            nc.vector.tensor_scalar(
                                oh[:], iota_f[:], cols_sb[:, ch:ch + 1],
                                None, op0=ALU.is_equal)
                            nc.tensor.matmul(
                                ps[:], oh[:], ecv[:, jj * FE:(jj + 1) * FE],
                                start=(j0 + jj == 0),
                                stop=(j0 + jj == CHG - 1))
                    # mean = sum / max(cnt, 1), node-major
                    rec = p_small.tile([128, 1], F32)
                    nc.vector.tensor_scalar(rec[:], ps[:, F:F + 1], 1.0, None,
                                            op0=ALU.max)
                    nc.vector.reciprocal(rec[:], rec[:])
                    agg = p_small.tile([128, F], F32, tag="agg")
                    nc.vector.tensor_scalar(agg[:], ps[:, 0:F], rec[:], None,
                                            op0=ALU.mult)
                    # transpose to feature-major and land in h0
                    for half in range(K1 // 2):
                        pst = ps_pst.tile([128, 128], F32)
                        nc.tensor.transpose(
                            pst[:], agg[:, half * 128:(half + 1) * 128],
                            ident[:])
                        nc.vector.tensor_copy(
                            h0_tiles[b][half][:, gi * 128:(gi + 1) * 128],
                            pst[:])

            def emit_x_conv(b):
                for kx in range(K1 // 2):
                    st = p_estage.tile([128, R], F32, tag="xstage")
                    nc.sync.dma_start(st[:], xT[kx, :, b * R:(b + 1) * R])
                    t = p_h0.tile([128, R], F32R, tag=f"h0x{kx}")
                    nc.vector.tensor_copy(t[:], st[:])
                    h0_tiles[b][K1 // 2 + kx] = t

            def emit_gemm(b, w_dram, k_tiles, nq, biases, relu, out_pool,
                          out_tag, out_dtype=F32R, qw=QW):
                """out[m] = act(sum_k w[k][:,m].T @ k_tiles[k] + bias[m])."""
                nk = len(k_tiles)
                nm_per_q = qw // 128
                outs = []
                for q in range(nq):
                    pss = [ps_gemm.tile([128, R], F32, name=f"psg{i}",
                                        tag=f"psg{i}")
                           for i in range(nm_per_q)]
                    for k in range(nk):
                        wcv = p_wconv.tile([128, qw], F32R)
                        nc.sync.dma_start(wcv[:], w_dram[k, q])
                        for m4 in range(nm_per_q):
                            nc.tensor.matmul(
                                pss[m4][:],
                                wcv[:, m4 * 128:(m4 + 1) * 128],
                                k_tiles[k][:],
                                start=(k == 0), stop=(k == nk - 1))
                    for m4 in range(nm_per_q):
                        m = q * nm_per_q + m4
                        if relu:
                            o = out_pool.tile([128, R], out_dtype,
                                              tag=out_tag)
                            nc.scalar.activation(o[:], pss[m4][:], AF.Relu,
                                                 bias=biases[:, m:m + 1])
                        else:
                            o = out_pool.tile([128, R], out_dtype,
                                              tag=out_tag)
                            nc.vector.tensor_scalar(
                                o[:], pss[m4][:], biases[:, m:m + 1], None,
                                op0=ALU.add)
                        outs.append(o)
                return outs

            def emit_ln(b, h4):
                """LayerNorm over features (partition dim), feature-major."""
                ps_s = ps_ln.tile([1, R], F32, tag="ln")
                for k in range(MT):
                    nc.tensor.matmul(ps_s[:], ones_r[:], h4[k][:],
                                     start=(k == 0), stop=(k == MT - 1))
                mu = p_lnrow.tile([1, R], F32)
                nc.scalar.mul(mu[:], ps_s[:], 1.0 / D)
                sqs = []
                for k in range(MT):
                    sq = p_sq.tile([128, R], F32R)
                    nc.vector.tensor_mul(sq[:], h4[k][:], h4[k][:])
                    sqs.append(sq)
                ps_q = ps_ln.tile([1, R], F32, tag="ln", name="ps_q")
                for k in range(MT):
                    nc.tensor.matmul(ps_q[:], ones_r[:], sqs[k][:],
                                     start=(k == 0), stop=(k == MT - 1))
                e2 = p_lnrow.tile([1, R], F32)
                nc.scalar.mul(e2[:], ps_q[:], 1.0 / D)
                mu2 = p_lnrow.tile([1, R], F32)
                nc.vector.tensor_mul(mu2[:], mu[:], mu[:])
                nc.vector.tensor_sub(e2[:], e2[:], mu2[:])
                sd = p_lnrow.tile([1, R], F32, tag="mu2", name="sd")
                nc.scalar.activation(sd[:], e2[:], AF.Sqrt, bias=eps_t[:])
                rstd = p_lnrow.tile([1, R], F32, tag="rstd")
                nc.vector.reciprocal(rstd[:], sd[:])
                mu_b = p_lnb.tile([128, R], F32)
                nc.gpsimd.partition_broadcast(mu_b[:], mu[:])
                rstd_b = p_lnb.tile([128, R], F32)
                nc.gpsimd.partition_broadcast(rstd_b[:], rstd[:])
                h5 = []
                for k in range(MT):
                    t1 = p_sq.tile([128, R], F32, tag="lnt1")
                    nc.vector.tensor_sub(t1[:], h4[k][:], mu_b[:])
                    t2 = p_sq.tile([128, R], F32, tag="lnt2")
                    nc.vector.tensor_mul(t2[:], t1[:], rstd_b[:])
                    o = p_h2.tile([128, R], F32R, tag="h2")
                    nc.vector.tensor_scalar(o[:], t2[:], gns[:, k:k + 1],
                                            bns[:, k:k + 1],
                                            op0=ALU.mult, op1=ALU.add)
                    h5.append(o)
                return h5

            def emit_block(b):
                emit_x_conv(b)
                h2 = emit_gemm(b, W1t, h0_tiles[b], Q, b1s, True, p_h2, "h2")
                h4 = emit_gemm(b, W3t, h2, Q, b3s, True, p_h4, "h4")
                h5 = emit_ln(b, h4)
                h7 = emit_gemm(b, W6t, h5 + h0_tiles[b], Q, b6s, True,
                               p_h4, "h4")
                o = emit_gemm(b, W8t, h7, 1, b8s, False, p_outsb, "o",
                              out_dtype=F32, qw=DO)
                for m in range(MO):
                    nc.sync.dma_start(
                        outT[m * 128:(m + 1) * 128, b * R:(b + 1) * R],
                        o[m][:])

            # ---------------- schedule: scatter one block ahead of GEMMs
            emit_scatter_block(0)
            for b in range(n_blks):
                if b + 1 < n_blks:
                    emit_scatter_block(b + 1)
                emit_block(b)

    nc.compile()
    return nc


# ------------------------------------------------------------- host prep

def host_prep_core(cfg, c, col, edge_attr, x):
    """Build per-core input arrays.  col is the (int64) destination array."""
    rows, chunks, n_blks = _derived(cfg)
    F = cfg["f"]
    FE = F + 2
    NPC = cfg["nodes_pc"]
    G = cfg["groups"]
    CHG = cfg["ch_g"]
    slots_g = CHG * 128

    lo, hi = c * NPC, (c + 1) * NPC
    idx = np.nonzero((col >= lo) & (col < hi))[0]
    lc = (col[idx] - lo).astype(np.int64)
    g = lc >> 7
    cnt_g = np.bincount(g, minlength=G)
    if cnt_g.max() > slots_g:
        raise RuntimeError(
            f"group overflow: {cnt_g.max()} > {slots_g}; raise ch_g")
    order = np.argsort(g, kind="stable")
    g_sorted = g[order]
    starts = np.zeros(G, dtype=np.int64)
    starts[1:] = np.cumsum(cnt_g)[:-1]
    rank = np.arange(len(order)) - starts[g_sorted]
    dst = g_sorted * slots_g + rank

    ebuf = np.zeros((chunks * 128, FE), dtype=np.float32)
    ebuf[dst, :F] = edge_attr[idx[order]]
    ebuf[dst, F] = 1.0
    cbuf = np.full(chunks * 128, -1.0, dtype=np.float32)
    cbuf[dst] = (lc[order] & 127).astype(np.float32)

    edges_t = np.ascontiguousarray(
        ebuf.reshape(chunks, 128, FE).transpose(1, 0, 2)
    ).reshape(128, chunks * FE)
    cols_t = np.ascontiguousarray(cbuf.reshape(chunks, 128).T)

    xpad = np.zeros((rows, F), dtype=np.float32)
    xpad[:NPC] = x[lo:hi]
    xT = np.ascontiguousarray(xpad.T).reshape(F // 128, 128, rows)
    return edges_t, cols_t, xT


def _round_f32r(x):
    """Round fp32 to the float32r grid (RNE, 11 explicit mantissa bits) --
    bit-exact to the on-device DVE cast, so weights can ship pre-rounded."""
    xi = np.asarray(x, np.float32).view(np.uint32).astype(np.int64)
    bias = ((xi >> 12) & 1) + 0x7FF
    xi = ((xi + bias) >> 12 << 12) & 0xFFFFFFFF
    return xi.astype(np.uint32).view(np.float32)


def _wtile(W, qw=512):
    """[K, Dout] -> [K/128, Dout/qw, 128, qw] contiguous stream tiles."""
    K, Do = W.shape
    W = _round_f32r(W)
    return np.ascontiguousarray(
        W.reshape(K // 128, 128, Do // qw, qw).transpose(0, 2, 1, 3))


def _btile(v):
    """[D] -> [128, D/128] with t[p, m] = v[m*128 + p]."""
    return np.ascontiguousarray(v.reshape(-1, 128).T)


def host_prep(cfg, x, edge_index, edge_attr, W1, b1, W3, b3, ln_g, ln_b,
              W6, b6, W8, b8):
    col = np.asarray(edge_index[1]).astype(np.int64)
    x = np.asarray(x, dtype=np.float32)
    edge_attr = np.asarray(edge_attr, dtype=np.float32)
    shared = dict(
        W1t=_wtile(np.asarray(W1, np.float32)),
        W3t=_wtile(np.asarray(W3, np.float32)),
        W6t=_wtile(np.asarray(W6, np.float32)),
        W8t=_wtile(np.asarray(W8, np.float32), qw=cfg["d_out"]),
        b1t=_btile(np.asarray(b1, np.float32)),
        b3t=_btile(np.asarray(b3, np.float32)),
        b6t=_btile(np.asarray(b6, np.float32)),
        b8t=_btile(np.asarray(b8, np.float32)),
        gnt=_btile(np.asarray(ln_g, np.float32)),
        bnt=_btile(np.asarray(ln_b, np.float32)),
    )
    in_maps = []
    for c in range(cfg["n_cores"]):
        edges_t, cols_t, xT = host_prep_core(cfg, c, col, edge_attr, x)
        m = dict(shared)
        m.update(edges_t=edges_t, cols_t=cols_t, xT=xT)
        in_maps.append(m)
    return in_maps


def assemble_output(cfg, results):
    NPC = cfg["nodes_pc"]
    DO = cfg["d_out"]
    out = np.empty((cfg["n_cores"] * NPC, DO), dtype=np.float32)
    for c, res in enumerate(results):
        out[c * NPC:(c + 1) * NPC] = res["outT"][:, :NPC].T
    return out


# ------------------------------------------------------------ entry point

_CACHE = {}


def _get_program(cfg_key):
    if cfg_key not in _CACHE:
        _CACHE[cfg_key] = build_program(FULL_CFG)
    return _CACHE[cfg_key]


def kernel(x, edge_index, edge_attr, u_counts=None, nb=None, eb=None,
           W1=None, b1=None, W3=None, b3=None, ln_g=None, ln_b=None,
           W6=None, b6=None, W8=None, b8=None, **_unused):
    cfg = FULL_CFG
    nc = _get_program("full")
    in_maps = host_prep(cfg, x, edge_index, edge_attr, W1, b1, W3, b3,
                        ln_g, ln_b, W6, b6, W8, b8)
    res = run_bass_kernel_spmd(nc, in_maps, list(range(cfg["n_cores"])))
    return assemble_output(cfg, res.results)


# revision 15
# speedup vs baseline: 1.1504x; 1.1504x over previous
"""Trainium2 Bass kernel for nn_MergeModel (GNN message passing + MLP chain).

Strategy (8 NeuronCores, SPMD, no collectives):
  - Nodes are partitioned into 8 contiguous ranges of 2500 (one per core).
  - Edges are partitioned by destination node (edge_index[1]), so each
    core's scatter_mean is fully local.  Host-side prep buckets edges by
    128-node destination group and pads each group to a fixed number of
    128-edge chunks (SPMD requires uniform shapes).
  - Scatter-sum runs on the TensorEngine: for each 128-edge chunk a
    one-hot matrix (built on-device with iota + is_equal) is matmul'd
    against [edge_attr | 1] to accumulate per-node sums and counts in
    PSUM.  Division by max(count,1) gives the mean; a PE transpose puts
    the result in feature-major layout.
  - The 4-GEMM MLP chain runs feature-major (activations stored
    transposed, [features, rows]) so no transposes are needed between
    layers.  LayerNorm partition reductions are done with ones-vector
    matmuls on the PE; mean/rstd rows are broadcast across partitions
    with a GpSimd partition_broadcast.
  - All matmuls use float32r (TF32-like, 1 cycle/row for N>=256; ~16x
    more accurate than bf16).
  - Output is written feature-major [512, rows] and transposed on the
    host during unsharding.
"""

import numpy as np

import concourse.bass as bass
import concourse.tile as tile
from concourse import bacc, mybir
from concourse.bass_utils import run_bass_kernel_spmd
from concourse import masks

F32 = mybir.dt.float32
F32R = mybir.dt.float32r
AF = mybir.ActivationFunctionType
ALU = mybir.AluOpType

# ---------------------------------------------------------------- config

FULL_CFG = dict(
    n_cores=8,
    nodes_pc=2500,     # real nodes per core
    groups=20,         # 128-node groups per core (20*128 = 2560 slots)
    ch_g=36,           # 128-edge chunks per group (36*128 = 4608 edge slots)
    f=256,             # node/edge feature dim
    d=2048,            # hidden width
    d_out=512,         # final out dim
    r_blk=512,         # rows per GEMM block
    eps=1e-5,
    edge_batch=4,      # chunks per edge DMA
)


def _derived(cfg):
    rows = cfg["groups"] * 128
    chunks = cfg["groups"] * cfg["ch_g"]
    n_blks = rows // cfg["r_blk"]
    assert rows % cfg["r_blk"] == 0
    assert cfg["ch_g"] % cfg["edge_batch"] == 0
    return rows, chunks, n_blks


# ---------------------------------------------------------- device program

def build_program(cfg):
    rows, chunks, n_blks = _derived(cfg)
    F = cfg["f"]
    D = cfg["d"]
    DO = cfg["d_out"]
    R = cfg["r_blk"]
    G = cfg["groups"]
    CHG = cfg["ch_g"]
    EB = cfg["edge_batch"]
    FE = F + 2                      # edge features + ones col + pad (fp32r needs even N)
    MT = D // 128                   # m-tiles of hidden dim
    MO = DO // 128                  # m-tiles of out dim
    K1 = (2 * F) // 128             # k-tiles of h0 (agg + x)
    K3 = MT + K1                    # k-tiles of GEMM3 input (h5 | h0)
    QW = 512                        # weight-stream column width
    Q = D // QW                     # q-passes over hidden out dim
    GPB = R // 128                  # groups per block

    nc = bacc.Bacc("TRN2", target_bir_lowering=False, debug=False,
                   num_devices=cfg["n_cores"])

    edges_t = nc.dram_tensor("edges_t", [128, chunks * FE], F32,
                             kind="ExternalInput").ap()
    cols_t = nc.dram_tensor("cols_t", [128, chunks], F32,
                            kind="ExternalInput").ap()
    xT = nc.dram_tensor("xT", [K1 // 2, 128, rows], F32,
                        kind="ExternalInput").ap()
    W1t = nc.dram_tensor("W1t", [K1, Q, 128, QW], F32R,
                         kind="ExternalInput").ap()
    W3t = nc.dram_tensor("W3t", [MT, Q, 128, QW], F32R,
                         kind="ExternalInput").ap()
    W6t = nc.dram_tensor("W6t", [K3, Q, 128, QW], F32R,
                         kind="ExternalInput").ap()
    W8t = nc.dram_tensor("W8t", [MT, 1, 128, DO], F32R,
                         kind="ExternalInput").ap()
    b1t = nc.dram_tensor("b1t", [128, MT], F32, kind="ExternalInput").ap()
    b3t = nc.dram_tensor("b3t", [128, MT], F32, kind="ExternalInput").ap()
    b6t = nc.dram_tensor("b6t", [128, MT], F32, kind="ExternalInput").ap()
    b8t = nc.dram_tensor("b8t", [128, MO], F32, kind="ExternalInput").ap()
    gnt = nc.dram_tensor("gnt", [128, MT], F32, kind="ExternalInput").ap()
    bnt = nc.dram_tensor("bnt", [128, MT], F32, kind="ExternalInput").ap()
    outT = nc.dram_tensor("outT", [DO, rows], F32, kind="ExternalOutput").ap()

    with tile.TileContext(nc) as tc:
        with (
            tc.tile_pool(name="const", bufs=1) as p_const,
            tc.tile_pool(name="h0", bufs=n_blks) as p_h0,
            tc.tile_pool(name="h2h5", bufs=MT + 4) as p_h2,
            tc.tile_pool(name="h4h7", bufs=MT + 4) as p_h4,
            tc.tile_pool(name="estage", bufs=3) as p_estage,
            tc.tile_pool(name="econv", bufs=3) as p_econv,
            tc.tile_pool(name="oh", bufs=8) as p_oh,
            tc.tile_pool(name="wconv", bufs=6) as p_wconv,
            tc.tile_pool(name="small", bufs=2) as p_small,
            tc.tile_pool(name="lnrow", bufs=1) as p_lnrow,
            tc.tile_pool(name="lnb", bufs=1) as p_lnb,
            tc.tile_pool(name="sq", bufs=3) as p_sq,
            tc.tile_pool(name="outsb", bufs=2) as p_outsb,
            tc.tile_pool(name="ps_sc", bufs=2, space="PSUM") as ps_sc,
            tc.tile_pool(name="ps_gemm", bufs=1, space="PSUM") as ps_gemm,
            tc.tile_pool(name="ps_pst", bufs=1, space="PSUM") as ps_pst,
            tc.tile_pool(name="ps_ln", bufs=1, space="PSUM") as ps_ln,
        ):
            # ---------------- constants
            iota_i = p_const.tile([128, 128], mybir.dt.int32)
            nc.gpsimd.iota(iota_i[:], pattern=[[1, 128]], base=0,
                           channel_multiplier=0)
            iota_f = p_const.tile([128, 128], F32)
            nc.vector.tensor_copy(iota_f[:], iota_i[:])
            ident = p_const.tile([128, 128], F32)
            masks.make_identity(nc, ident[:])
            ones_f = p_const.tile([128, 1], F32)
            nc.gpsimd.memset(ones_f[:], 1.0)
            ones_r = p_const.tile([128, 1], F32R)
            nc.vector.tensor_copy(ones_r[:], ones_f[:])
            eps_t = p_const.tile([1, 1], F32)
            nc.gpsimd.memset(eps_t[:], cfg["eps"])

            cols_sb = p_const.tile([128, chunks], F32)
            nc.sync.dma_start(cols_sb[:], cols_t)
            b1s = p_const.tile([128, MT], F32)
            nc.sync.dma_start(b1s[:], b1t)
            b3s = p_const.tile([128, MT], F32)
            nc.sync.dma_start(b3s[:], b3t)
            b6s = p_const.tile([128, MT], F32)
            nc.sync.dma_start(b6s[:], b6t)
            b8s = p_const.tile([128, MO], F32)
            nc.sync.dma_start(b8s[:], b8t)
            gns = p_const.tile([128, MT], F32)
            nc.sync.dma_start(gns[:], gnt)
            bns = p_const.tile([128, MT], F32)
            nc.sync.dma_start(bns[:], bnt)

            h0_tiles = [[None] * K1 for _ in range(n_blks)]

            # ---------------- phase emitters
            def emit_scatter_block(b):
                """Scatter-mean for groups [b*GPB, (b+1)*GPB) -> h0 tiles."""
                for half in range(K1 // 2):
                    t = p_h0.tile([128, R], F32R, tag=f"h0a{half}")
                    h0_tiles[b][half] = t
                for gi in range(GPB):
                    g = b * GPB + gi
                    ps = ps_sc.tile([128, FE], F32)
                    for j0 in range(0, CHG, EB):
                        est = p_estage.tile([128, EB * FE], F32)
                        ch0 = g * CHG + j0
                        nc.sync.dma_start(
                            est[:], edges_t[:, ch0 * FE:(ch0 + EB) * FE])
                        ecv = p_econv.tile([128, EB * FE], F32R)
                        nc.vector.tensor_copy(ecv[:], est[:])
                        for jj in range(EB):
                            ch = ch0 + jj
                            oh = p_oh.tile([128, 128], F32R)
                            nc.vector.tensor_scalar(
                                oh[:], iota_f[:], cols_sb[:, ch:ch + 1],
                                None, op0=ALU.is_equal)
                            nc.tensor.matmul(
                                ps[:], oh[:], ecv[:, jj * FE:(jj + 1) * FE],
                                start=(j0 + jj == 0),
                                stop=(j0 + jj == CHG - 1))
                    # mean = sum / max(cnt, 1), node-major
                    rec = p_small.tile([128, 1], F32)
                    nc.vector.tensor_scalar(rec[:], ps[:, F:F + 1], 1.0, None,
                                            op0=ALU.max)
                    nc.vector.reciprocal(rec[:], rec[:])
                    agg = p_small.tile([128, F], F32, tag="agg")
                    nc.vector.tensor_scalar(agg[:], ps[:, 0:F], rec[:], None,
                                            op0=ALU.mult)
                    # transpose to feature-major and land in h0
                    for half in range(K1 // 2):
                        pst = ps_pst.tile([128, 128], F32)
                        nc.tensor.transpose(
                            pst[:], agg[:, half * 128:(half + 1) * 128],
                            ident[:])
                        nc.vector.tensor_copy(
                            h0_tiles[b][half][:, gi * 128:(gi + 1) * 128],
                            pst[:])

            def emit_x_conv(b):
                for kx in range(K1 // 2):
                    st = p_estage.tile([128, R], F32, tag="xstage")
                    nc.sync.dma_start(st[:], xT[kx, :, b * R:(b + 1) * R])
                    t = p_h0.tile([128, R], F32R, tag=f"h0x{kx}")
                    nc.vector.tensor_copy(t[:], st[:])
                    h0_tiles[b][K1 // 2 + kx] = t

            def emit_gemm(b, w_dram, k_tiles, nq, biases, relu, out_pool,
                          out_tag, out_dtype=F32R, qw=QW):
                """out[m] = act(sum_k w[k][:,m].T @ k_tiles[k] + bias[m])."""
                nk = len(k_tiles)
                nm_per_q = qw // 128
                outs = []
                for q in range(nq):
                    pss = [ps_gemm.tile([128, R], F32, name=f"psg{i}",
                                        tag=f"psg{i}")
                           for i in range(nm_per_q)]
                    for k in range(nk):
                        wcv = p_wconv.tile([128, qw], F32R)
                        nc.sync.dma_start(wcv[:], w_dram[k, q])
                        for m4 in range(nm_per_q):
                            nc.tensor.matmul(
                                pss[m4][:],
                                wcv[:, m4 * 128:(m4 + 1) * 128],
                                k_tiles[k][:],
                                start=(k == 0), stop=(k == nk - 1))
                    for m4 in range(nm_per_q):
                        m = q * nm_per_q + m4
                        if relu:
                            o = out_pool.tile([128, R], out_dtype,
                                              tag=out_tag)
                            nc.scalar.activation(o[:], pss[m4][:], AF.Relu,
                                                 bias=biases[:, m:m + 1])
                        else:
                            o = out_pool.tile([128, R], out_dtype,
                                              tag=out_tag)
                            nc.vector.tensor_scalar(
                                o[:], pss[m4][:], biases[:, m:m + 1], None,
                                op0=ALU.add)
                        outs.append(o)
                return outs

            def emit_ln(b, h4):
                """LayerNorm over features (partition dim), feature-major."""
                ps_s = ps_ln.tile([1, R], F32, tag="ln")
                for k in range(MT):
                    nc.tensor.matmul(ps_s[:], ones_r[:], h4[k][:],
                                     start=(k == 0), stop=(k == MT - 1))
                mu = p_lnrow.tile([1, R], F32)
                nc.scalar.mul(mu[:], ps_s[:], 1.0 / D)
                sqs = []
                for k in range(MT):
                    sq = p_sq.tile([128, R], F32R)
                    nc.vector.tensor_mul(sq[:], h4[k][:], h4[k][:])
                    sqs.append(sq)
                ps_q = ps_ln.tile([1, R], F32, tag="ln", name="ps_q")
                for k in range(MT):
                    nc.tensor.matmul(ps_q[:], ones_r[:], sqs[k][:],
                                     start=(k == 0), stop=(k == MT - 1))
                e2 = p_lnrow.tile([1, R], F32)
                nc.scalar.mul(e2[:], ps_q[:], 1.0 / D)
                mu2 = p_lnrow.tile([1, R], F32)
                nc.vector.tensor_mul(mu2[:], mu[:], mu[:])
                nc.vector.tensor_sub(e2[:], e2[:], mu2[:])
                sd = p_lnrow.tile([1, R], F32, tag="mu2", name="sd")
                nc.scalar.activation(sd[:], e2[:], AF.Sqrt, bias=eps_t[:])
                rstd = p_lnrow.tile([1, R], F32, tag="rstd")
                nc.vector.reciprocal(rstd[:], sd[:])
                mu_b = p_lnb.tile([128, R], F32)
                nc.gpsimd.partition_broadcast(mu_b[:], mu[:])
                rstd_b = p_lnb.tile([128, R], F32)
                nc.gpsimd.partition_broadcast(rstd_b[:], rstd[:])
                h5 = []
                for k in range(MT):
                    t1 = p_sq.tile([128, R], F32, tag="lnt1")
                    nc.vector.tensor_sub(t1[:], h4[k][:], mu_b[:])
                    t2 = p_sq.tile([128, R], F32, tag="lnt2")
                    nc.vector.tensor_mul(t2[:], t1[:], rstd_b[:])
                    o = p_h2.tile([128, R], F32R, tag="h2")
                    nc.vector.tensor_scalar(o[:], t2[:], gns[:, k:k + 1],
                                            bns[:, k:k + 1],
                                            op0=ALU.mult, op1=ALU.add)
                    h5.append(o)
                return h5

            def emit_block(b):
                emit_x_conv(b)
                h2 = emit_gemm(b, W1t, h0_tiles[b], Q, b1s, True, p_h2, "h2")
                h4 = emit_gemm(b, W3t, h2, Q, b3s, True, p_h4, "h4")
                h5 = emit_ln(b, h4)
                h7 = emit_gemm(b, W6t, h5 + h0_tiles[b], Q, b6s, True,
                               p_h4, "h4")
                o = emit_gemm(b, W8t, h7, 1, b8s, False, p_outsb, "o",
                              out_dtype=F32, qw=DO)
                for m in range(MO):
                    nc.sync.dma_start(
                        outT[m * 128:(m + 1) * 128, b * R:(b + 1) * R],
                        o[m][:])

            # ---------------- schedule: scatter one block ahead of GEMMs
            emit_scatter_block(0)
            for b in range(n_blks):
                if b + 1 < n_blks:
                    emit_scatter_block(b + 1)
                emit_block(b)

    nc.compile()
    return nc


# ------------------------------------------------------------- host prep

def host_prep_core(cfg, c, col, edge_attr, x):
    """Build per-core input arrays.  col is the (int64) destination array."""
    rows, chunks, n_blks = _derived(cfg)
    F = cfg["f"]
    FE = F + 2
    NPC = cfg["nodes_pc"]
    G = cfg["groups"]
    CHG = cfg["ch_g"]
    slots_g = CHG * 128

    lo, hi = c * NPC, (c + 1) * NPC
    idx = np.nonzero((col >= lo) & (col < hi))[0]
    lc = (col[idx] - lo).astype(np.int64)
    g = lc >> 7
    cnt_g = np.bincount(g, minlength=G)
    if cnt_g.max() > slots_g:
        raise RuntimeError(
            f"group overflow: {cnt_g.max()} > {slots_g}; raise ch_g")
    order = np.argsort(g, kind="stable")
    g_sorted = g[order]
    starts = np.zeros(G, dtype=np.int64)
    starts[1:] = np.cumsum(cnt_g)[:-1]
    rank = np.arange(len(order)) - starts[g_sorted]
    dst = g_sorted * slots_g + rank

    ebuf = np.zeros((chunks * 128, FE), dtype=np.float32)
    ebuf[dst, :F] = edge_attr[idx[order]]
    ebuf[dst, F] = 1.0
    cbuf = np.full(chunks * 128, -1.0, dtype=np.float32)
    cbuf[dst] = (lc[order] & 127).astype(np.float32)

    edges_t = np.ascontiguousarray(
        ebuf.reshape(chunks, 128, FE).transpose(1, 0, 2)
    ).reshape(128, chunks * FE)
    cols_t = np.ascontiguousarray(cbuf.reshape(chunks, 128).T)

    xpad = np.zeros((rows, F), dtype=np.float32)
    xpad[:NPC] = x[lo:hi]
    xT = np.ascontiguousarray(xpad.T).reshape(F // 128, 128, rows)
    return edges_t, cols_t, xT


def _round_f32r(x):
    """Round fp32 to the float32r grid (RNE, 11 explicit mantissa bits) --
    bit-exact to the on-device DVE cast, so weights can ship pre-rounded."""
    xi = np.asarray(x, np.float32).view(np.uint32).astype(np.int64)
    bias = ((xi >> 12) & 1) + 0x7FF
    xi = ((xi + bias) >> 12 << 12) & 0xFFFFFFFF
    return xi.astype(np.uint32).view(np.float32)


def _wtile(W, qw=512):
    """[K, Dout] -> [K/128, Dout/qw, 128, qw] contiguous stream tiles."""
    K, Do = W.shape
    W = _round_f32r(W)
    return np.ascontiguousarray(
        W.reshape(K // 128, 128, Do // qw, qw).transpose(0, 2, 1, 3))


def _btile(v):
    """[D] -> [128, D/128] with t[p, m] = v[m*128 + p]."""
    return np.ascontiguousarray(v.reshape(-1, 128).T)


def host_prep(cfg, x, edge_index, edge_attr, W1, b1, W3, b3, ln_g, ln_b,
              W6, b6, W8, b8):
    col = np.asarray(edge_index[1]).astype(np.int64)
    x = np.asarray(x, dtype=np.float32)
    edge_attr = np.asarray(edge_attr, dtype=np.float32)
    shared = dict(
        W1t=_wtile(np.asarray(W1, np.float32)),
        W3t=_wtile(np.asarray(W3, np.float32)),
        W6t=_wtile(np.asarray(W6, np.float32)),
        W8t=_wtile(np.asarray(W8, np.float32), qw=cfg["d_out"]),
        b1t=_btile(np.asarray(b1, np.float32)),
        b3t=_btile(np.asarray(b3, np.float32)),
        b6t=_btile(np.asarray(b6, np.float32)),
        b8t=_btile(np.asarray(b8, np.float32)),
        gnt=_btile(np.asarray(ln_g, np.float32)),
        bnt=_btile(np.asarray(ln_b, np.float32)),
    )
    in_maps = []
    for c in range(cfg["n_cores"]):
        edges_t, cols_t, xT = host_prep_core(cfg, c, col, edge_attr, x)
        m = dict(shared)
        m.update(edges_t=edges_t, cols_t=cols_t, xT=xT)
        in_maps.append(m)
    return in_maps


def assemble_output(cfg, results):
    NPC = cfg["nodes_pc"]
    DO = cfg["d_out"]
    out = np.empty((cfg["n_cores"] * NPC, DO), dtype=np.float32)
    for c, res in enumerate(results):
        out[c * NPC:(c + 1) * NPC] = res["outT"][:, :NPC].T
    return out


# ------------------------------------------------------------ entry point

_CACHE = {}


def _get_program(cfg_key):
    if cfg_key not in _CACHE:
        _CACHE[cfg_key] = build_program(FULL_CFG)
    return _CACHE[cfg_key]


def kernel(x, edge_index, edge_attr, u_counts=None, nb=None, eb=None,
           W1=None, b1=None, W3=None, b3=None, ln_g=None, ln_b=None,
           W6=None, b6=None, W8=None, b8=None, **_unused):
    cfg = FULL_CFG
    nc = _get_program("full")
    in_maps = host_prep(cfg, x, edge_index, edge_attr, W1, b1, W3, b3,
                        ln_g, ln_b, W6, b6, W8, b8)
    res = run_bass_kernel_spmd(nc, in_maps, list(range(cfg["n_cores"])))
    return assemble_output(cfg, res.results)


# revision 16
# speedup vs baseline: 1.2840x; 1.1161x over previous
"""Trainium2 Bass kernel for nn_MergeModel (GNN message passing + MLP chain).

Strategy (8 NeuronCores, SPMD, no collectives):
  - Nodes are partitioned into 8 contiguous ranges of 2500 (one per core).
  - Edges are partitioned by destination node (edge_index[1]), so each
    core's scatter_mean is fully local.  Host-side prep buckets edges by
    128-node destination group and pads each group to a fixed number of
    128-edge chunks (SPMD requires uniform shapes).
  - Scatter-sum runs on the TensorEngine: for each 128-edge chunk a
    one-hot matrix (built on-device with iota + is_equal) is matmul'd
    against [edge_attr | 1] to accumulate per-node sums and counts in
    PSUM.  Division by max(count,1) gives the mean; a PE transpose puts
    the result in feature-major layout.
  - The 4-GEMM MLP chain runs feature-major (activations stored
    transposed, [features, rows]) so no transposes are needed between
    layers.  LayerNorm partition reductions are done with ones-vector
    matmuls on the PE; mean/rstd rows are broadcast across partitions
    with a GpSimd partition_broadcast.
  - All matmuls use float32r (TF32-like, 1 cycle/row for N>=256; ~16x
    more accurate than bf16).
  - Output is written feature-major [512, rows] and transposed on the
    host during unsharding.
"""

import numpy as np

import concourse.bass as bass
import concourse.tile as tile
from concourse import bacc, mybir
from concourse.bass_utils import run_bass_kernel_spmd
from concourse import masks

F32 = mybir.dt.float32
F32R = mybir.dt.float32r
AF = mybir.ActivationFunctionType
ALU = mybir.AluOpType

# ---------------------------------------------------------------- config

FULL_CFG = dict(
    n_cores=8,
    nodes_pc=2500,     # real nodes per core
    groups=20,         # 128-node groups per core (20*128 = 2560 slots)
    ch_g=36,           # 128-edge chunks per group (36*128 = 4608 edge slots)
    f=256,             # node/edge feature dim
    d=2048,            # hidden width
    d_out=512,         # final out dim
    r_blk=512,         # rows per GEMM block
    eps=1e-5,
    edge_batch=4,      # chunks per edge DMA
)


def _derived(cfg):
    rows = cfg["groups"] * 128
    chunks = cfg["groups"] * cfg["ch_g"]
    n_blks = rows // cfg["r_blk"]
    assert rows % cfg["r_blk"] == 0
    assert cfg["ch_g"] % cfg["edge_batch"] == 0
    return rows, chunks, n_blks


# ---------------------------------------------------------- device program

def build_program(cfg):
    rows, chunks, n_blks = _derived(cfg)
    F = cfg["f"]
    D = cfg["d"]
    DO = cfg["d_out"]
    R = cfg["r_blk"]
    G = cfg["groups"]
    CHG = cfg["ch_g"]
    EB = cfg["edge_batch"]
    FE = F + 2                      # edge features + ones col + pad (fp32r needs even N)
    MT = D // 128                   # m-tiles of hidden dim
    MO = DO // 128                  # m-tiles of out dim
    K1 = (2 * F) // 128             # k-tiles of h0 (agg + x)
    K3 = MT + K1                    # k-tiles of GEMM3 input (h5 | h0)
    QW = 512                        # weight-stream column width
    Q = D // QW                     # q-passes over hidden out dim
    GPB = R // 128                  # groups per block

    nc = bacc.Bacc("TRN2", target_bir_lowering=False, debug=False,
                   num_devices=cfg["n_cores"])

    edges_t = nc.dram_tensor("edges_t", [128, chunks * FE], F32,
                             kind="ExternalInput").ap()
    cols_t = nc.dram_tensor("cols_t", [128, chunks], F32,
                            kind="ExternalInput").ap()
    xT = nc.dram_tensor("xT", [K1 // 2, 128, rows], F32,
                        kind="ExternalInput").ap()
    W1t = nc.dram_tensor("W1t", [K1, Q, 128, QW], F32R,
                         kind="ExternalInput").ap()
    W3t = nc.dram_tensor("W3t", [MT, Q, 128, QW], F32R,
                         kind="ExternalInput").ap()
    W6t = nc.dram_tensor("W6t", [K3, Q, 128, QW], F32R,
                         kind="ExternalInput").ap()
    W8t = nc.dram_tensor("W8t", [MT, 1, 128, DO], F32R,
                         kind="ExternalInput").ap()
    b1t = nc.dram_tensor("b1t", [128, MT], F32, kind="ExternalInput").ap()
    b3t = nc.dram_tensor("b3t", [128, MT], F32, kind="ExternalInput").ap()
    b6t = nc.dram_tensor("b6t", [128, MT], F32, kind="ExternalInput").ap()
    b8t = nc.dram_tensor("b8t", [128, MO], F32, kind="ExternalInput").ap()
    gnt = nc.dram_tensor("gnt", [128, MT], F32, kind="ExternalInput").ap()
    bnt = nc.dram_tensor("bnt", [128, MT], F32, kind="ExternalInput").ap()
    outT = nc.dram_tensor("outT", [DO, rows], F32, kind="ExternalOutput").ap()

    with tile.TileContext(nc) as tc:
        with (
            tc.tile_pool(name="const", bufs=1) as p_const,
            tc.tile_pool(name="h0", bufs=n_blks) as p_h0,
            tc.tile_pool(name="h2h5", bufs=MT + 4) as p_h2,
            tc.tile_pool(name="h4h7", bufs=MT + 4) as p_h4,
            tc.tile_pool(name="estage", bufs=3) as p_estage,
            tc.tile_pool(name="econv", bufs=3) as p_econv,
            tc.tile_pool(name="oh", bufs=8) as p_oh,
            tc.tile_pool(name="wconv", bufs=6) as p_wconv,
            tc.tile_pool(name="small", bufs=2) as p_small,
            tc.tile_pool(name="lnrow", bufs=1) as p_lnrow,
            tc.tile_pool(name="lnb", bufs=1) as p_lnb,
            tc.tile_pool(name="sq", bufs=3) as p_sq,
            tc.tile_pool(name="outsb", bufs=2) as p_outsb,
            tc.tile_pool(name="ps_sc", bufs=2, space="PSUM") as ps_sc,
            tc.tile_pool(name="ps_gemm", bufs=1, space="PSUM") as ps_gemm,
            tc.tile_pool(name="ps_pst", bufs=1, space="PSUM") as ps_pst,
            tc.tile_pool(name="ps_ln", bufs=1, space="PSUM") as ps_ln,
        ):
            # ---------------- constants
            iota_i = p_const.tile([128, 128], mybir.dt.int32)
            nc.gpsimd.iota(iota_i[:], pattern=[[1, 128]], base=0,
                           channel_multiplier=0)
            iota_f = p_const.tile([128, 128], F32)
            nc.vector.tensor_copy(iota_f[:], iota_i[:])
            ident = p_const.tile([128, 128], F32)
            masks.make_identity(nc, ident[:])
            ones_f = p_const.tile([128, 1], F32)
            nc.gpsimd.memset(ones_f[:], 1.0)
            ones_r = p_const.tile([128, 1], F32R)
            nc.vector.tensor_copy(ones_r[:], ones_f[:])
            eps_t = p_const.tile([1, 1], F32)
            nc.gpsimd.memset(eps_t[:], cfg["eps"])

            cols_sb = p_const.tile([128, chunks], F32)
            nc.sync.dma_start(cols_sb[:], cols_t)
            b1s = p_const.tile([128, MT], F32)
            nc.sync.dma_start(b1s[:], b1t)
            b3s = p_const.tile([128, MT], F32)
            nc.sync.dma_start(b3s[:], b3t)
            b6s = p_const.tile([128, MT], F32)
            nc.sync.dma_start(b6s[:], b6t)
            b8s = p_const.tile([128, MO], F32)
            nc.sync.dma_start(b8s[:], b8t)
            gns = p_const.tile([128, MT], F32)
            nc.sync.dma_start(gns[:], gnt)
            bns = p_const.tile([128, MT], F32)
            nc.sync.dma_start(bns[:], bnt)

            h0_tiles = [[None] * K1 for _ in range(n_blks)]

            # ---------------- phase emitters
            def emit_scatter_block(b):
                """Scatter-mean for groups [b*GPB, (b+1)*GPB) -> h0 tiles."""
                for half in range(K1 // 2):
                    t = p_h0.tile([128, R], F32R, tag=f"h0a{half}")
                    h0_tiles[b][half] = t
                for gi in range(GPB):
                    g = b * GPB + gi
                    ps = ps_sc.tile([128, FE], F32)
                    for j0 in range(0, CHG, EB):
                        est = p_estage.tile([128, EB * FE], F32)
                        ch0 = g * CHG + j0
                        nc.sync.dma_start(
                            est[:], edges_t[:, ch0 * FE:(ch0 + EB) * FE])
                        ecv = p_econv.tile([128, EB * FE], F32R)
                        nc.vector.tensor_copy(ecv[:], est[:])
                        for jj in range(EB):
                            ch = ch0 + jj
                            oh = p_oh.tile([128, 128], F32R)
                            nc.vector.tensor_scalar(
                                oh[:], iota_f[:], cols_sb[:, ch:ch + 1],
                                None, op0=ALU.is_equal)
                            nc.tensor.matmul(
                                ps[:], oh[:], ecv[:, jj * FE:(jj + 1) * FE],
                                start=(j0 + jj == 0),
                                stop=(j0 + jj == CHG - 1))
                    # mean = sum / max(cnt, 1), node-major
                    rec = p_small.tile([128, 1], F32)
                    nc.vector.tensor_scalar(rec[:], ps[:, F:F + 1], 1.0, None,
                                            op0=ALU.max)
                    nc.vector.reciprocal(rec[:], rec[:])
                    agg = p_small.tile([128, F], F32, tag="agg")
                    nc.vector.tensor_scalar(agg[:], ps[:, 0:F], rec[:], None,
                                            op0=ALU.mult)
                    # transpose to feature-major and land in h0
                    for half in range(K1 // 2):
                        pst = ps_pst.tile([128, 128], F32)
                        nc.tensor.transpose(
                            pst[:], agg[:, half * 128:(half + 1) * 128],
                            ident[:])
                        nc.vector.tensor_copy(
                            h0_tiles[b][half][:, gi * 128:(gi + 1) * 128],
                            pst[:])

            def emit_x_conv(b):
                for kx in range(K1 // 2):
                    st = p_estage.tile([128, R], F32, tag="xstage")
                    nc.sync.dma_start(st[:], xT[kx, :, b * R:(b + 1) * R])
                    t = p_h0.tile([128, R], F32R, tag=f"h0x{kx}")
                    nc.vector.tensor_copy(t[:], st[:])
                    h0_tiles[b][K1 // 2 + kx] = t

            def emit_gemm(b, w_dram, k_tiles, nq, biases, relu, out_pool,
                          out_tag, out_dtype=F32R, qw=QW):
                """out[m] = act(sum_k w[k][:,m].T @ k_tiles[k] + bias[m])."""
                nk = len(k_tiles)
                nm_per_q = qw // 128
                outs = []
                for q in range(nq):
                    pss = [ps_gemm.tile([128, R], F32, name=f"psg{i}",
                                        tag=f"psg{i}")
                           for i in range(nm_per_q)]
                    for k in range(nk):
                        wcv = p_wconv.tile([128, qw], F32R)
                        nc.sync.dma_start(wcv[:], w_dram[k, q])
                        for m4 in range(nm_per_q):
                            nc.tensor.matmul(
                                pss[m4][:],
                                wcv[:, m4 * 128:(m4 + 1) * 128],
                                k_tiles[k][:],
                                start=(k == 0), stop=(k == nk - 1))
                    for m4 in range(nm_per_q):
                        m = q * nm_per_q + m4
                        if relu:
                            o = out_pool.tile([128, R], out_dtype,
                                              tag=out_tag)
                            nc.scalar.activation(o[:], pss[m4][:], AF.Relu,
                                                 bias=biases[:, m:m + 1])
                        else:
                            o = out_pool.tile([128, R], out_dtype,
                                              tag=out_tag)
                            nc.vector.tensor_scalar(
                                o[:], pss[m4][:], biases[:, m:m + 1], None,
                                op0=ALU.add)
                        outs.append(o)
                return outs

            def emit_ln(b, h4):
                """LayerNorm over features (partition dim), feature-major."""
                ps_s = ps_ln.tile([1, R], F32, tag="ln")
                for k in range(MT):
                    nc.tensor.matmul(ps_s[:], ones_r[:], h4[k][:],
                                     start=(k == 0), stop=(k == MT - 1))
                mu = p_lnrow.tile([1, R], F32)
                nc.scalar.mul(mu[:], ps_s[:], 1.0 / D)
                sqs = []
                for k in range(MT):
                    sq = p_sq.tile([128, R], F32R)
                    nc.scalar.activation(sq[:], h4[k][:], AF.Square)
                    sqs.append(sq)
                ps_q = ps_ln.tile([1, R], F32, tag="ln", name="ps_q")
                for k in range(MT):
                    nc.tensor.matmul(ps_q[:], ones_r[:], sqs[k][:],
                                     start=(k == 0), stop=(k == MT - 1))
                e2 = p_lnrow.tile([1, R], F32)
                nc.scalar.mul(e2[:], ps_q[:], 1.0 / D)
                mu2 = p_lnrow.tile([1, R], F32)
                nc.vector.tensor_mul(mu2[:], mu[:], mu[:])
                nc.vector.tensor_sub(e2[:], e2[:], mu2[:])
                sd = p_lnrow.tile([1, R], F32, tag="mu2", name="sd")
                nc.scalar.activation(sd[:], e2[:], AF.Sqrt, bias=eps_t[:])
                rstd = p_lnrow.tile([1, R], F32, tag="rstd")
                nc.vector.reciprocal(rstd[:], sd[:])
                mu_b = p_lnb.tile([128, R], F32)
                nc.gpsimd.partition_broadcast(mu_b[:], mu[:])
                rstd_b = p_lnb.tile([128, R], F32)
                nc.gpsimd.partition_broadcast(rstd_b[:], rstd[:])
                h5 = []
                for k in range(MT):
                    t1 = p_sq.tile([128, R], F32, tag="lnt1")
                    nc.vector.tensor_sub(t1[:], h4[k][:], mu_b[:])
                    t2 = p_sq.tile([128, R], F32, tag="lnt2")
                    nc.vector.tensor_mul(t2[:], t1[:], rstd_b[:])
                    o = p_h2.tile([128, R], F32R, tag="h2")
                    nc.vector.tensor_scalar(o[:], t2[:], gns[:, k:k + 1],
                                            bns[:, k:k + 1],
                                            op0=ALU.mult, op1=ALU.add)
                    h5.append(o)
                return h5

            def emit_block(b):
                emit_x_conv(b)
                h2 = emit_gemm(b, W1t, h0_tiles[b], Q, b1s, True, p_h2, "h2")
                h4 = emit_gemm(b, W3t, h2, Q, b3s, True, p_h4, "h4")
                h5 = emit_ln(b, h4)
                h7 = emit_gemm(b, W6t, h5 + h0_tiles[b], Q, b6s, True,
                               p_h4, "h4")
                o = emit_gemm(b, W8t, h7, 1, b8s, False, p_outsb, "o",
                              out_dtype=F32, qw=DO)
                for m in range(MO):
                    nc.sync.dma_start(
                        outT[m * 128:(m + 1) * 128, b * R:(b + 1) * R],
                        o[m][:])

            # ---------------- schedule: scatter one block ahead of GEMMs
            emit_scatter_block(0)
            for b in range(n_blks):
                if b + 1 < n_blks:
                    emit_scatter_block(b + 1)
                emit_block(b)

    nc.compile()
    return nc


# ------------------------------------------------------------- host prep

def host_prep_core(cfg, c, col, edge_attr, x):
    """Build per-core input arrays.  col is the (int64) destination array."""
    rows, chunks, n_blks = _derived(cfg)
    F = cfg["f"]
    FE = F + 2
    NPC = cfg["nodes_pc"]
    G = cfg["groups"]
    CHG = cfg["ch_g"]
    slots_g = CHG * 128

    lo, hi = c * NPC, (c + 1) * NPC
    idx = np.nonzero((col >= lo) & (col < hi))[0]
    lc = (col[idx] - lo).astype(np.int64)
    g = lc >> 7
    cnt_g = np.bincount(g, minlength=G)
    if cnt_g.max() > slots_g:
        raise RuntimeError(
            f"group overflow: {cnt_g.max()} > {slots_g}; raise ch_g")
    order = np.argsort(g, kind="stable")
    g_sorted = g[order]
    starts = np.zeros(G, dtype=np.int64)
    starts[1:] = np.cumsum(cnt_g)[:-1]
    rank = np.arange(len(order)) - starts[g_sorted]
    dst = g_sorted * slots_g + rank

    ebuf = np.zeros((chunks * 128, FE), dtype=np.float32)
    ebuf[dst, :F] = edge_attr[idx[order]]
    ebuf[dst, F] = 1.0
    cbuf = np.full(chunks * 128, -1.0, dtype=np.float32)
    cbuf[dst] = (lc[order] & 127).astype(np.float32)

    edges_t = np.ascontiguousarray(
        ebuf.reshape(chunks, 128, FE).transpose(1, 0, 2)
    ).reshape(128, chunks * FE)
    cols_t = np.ascontiguousarray(cbuf.reshape(chunks, 128).T)

    xpad = np.zeros((rows, F), dtype=np.float32)
    xpad[:NPC] = x[lo:hi]
    xT = np.ascontiguousarray(xpad.T).reshape(F // 128, 128, rows)
    return edges_t, cols_t, xT


def _round_f32r(x):
    """Round fp32 to the float32r grid (RNE, 11 explicit mantissa bits) --
    bit-exact to the on-device DVE cast, so weights can ship pre-rounded."""
    xi = np.asarray(x, np.float32).view(np.uint32).astype(np.int64)
    bias = ((xi >> 12) & 1) + 0x7FF
    xi = ((xi + bias) >> 12 << 12) & 0xFFFFFFFF
    return xi.astype(np.uint32).view(np.float32)


def _wtile(W, qw=512):
    """[K, Dout] -> [K/128, Dout/qw, 128, qw] contiguous stream tiles."""
    K, Do = W.shape
    W = _round_f32r(W)
    return np.ascontiguousarray(
        W.reshape(K // 128, 128, Do // qw, qw).transpose(0, 2, 1, 3))


def _btile(v):
    """[D] -> [128, D/128] with t[p, m] = v[m*128 + p]."""
    return np.ascontiguousarray(v.reshape(-1, 128).T)


def host_prep(cfg, x, edge_index, edge_attr, W1, b1, W3, b3, ln_g, ln_b,
              W6, b6, W8, b8):
    col = np.asarray(edge_index[1]).astype(np.int64)
    x = np.asarray(x, dtype=np.float32)
    edge_attr = np.asarray(edge_attr, dtype=np.float32)
    shared = dict(
        W1t=_wtile(np.asarray(W1, np.float32)),
        W3t=_wtile(np.asarray(W3, np.float32)),
        W6t=_wtile(np.asarray(W6, np.float32)),
        W8t=_wtile(np.asarray(W8, np.float32), qw=cfg["d_out"]),
        b1t=_btile(np.asarray(b1, np.float32)),
        b3t=_btile(np.asarray(b3, np.float32)),
        b6t=_btile(np.asarray(b6, np.float32)),
        b8t=_btile(np.asarray(b8, np.float32)),
        gnt=_btile(np.asarray(ln_g, np.float32)),
        bnt=_btile(np.asarray(ln_b, np.float32)),
    )
    in_maps = []
    for c in range(cfg["n_cores"]):
        edges_t, cols_t, xT = host_prep_core(cfg, c, col, edge_attr, x)
        m = dict(shared)
        m.update(edges_t=edges_t, cols_t=cols_t, xT=xT)
        in_maps.append(m)
    return in_maps


def assemble_output(cfg, results):
    NPC = cfg["nodes_pc"]
    DO = cfg["d_out"]
    out = np.empty((cfg["n_cores"] * NPC, DO), dtype=np.float32)
    for c, res in enumerate(results):
        out[c * NPC:(c + 1) * NPC] = res["outT"][:, :NPC].T
    return out


# ------------------------------------------------------------ entry point

_CACHE = {}


def _get_program(cfg_key):
    if cfg_key not in _CACHE:
        _CACHE[cfg_key] = build_program(FULL_CFG)
    return _CACHE[cfg_key]


def kernel(x, edge_index, edge_attr, u_counts=None, nb=None, eb=None,
           W1=None, b1=None, W3=None, b3=None, ln_g=None, ln_b=None,
           W6=None, b6=None, W8=None, b8=None, **_unused):
    cfg = FULL_CFG
    nc = _get_program("full")
    in_maps = host_prep(cfg, x, edge_index, edge_attr, W1, b1, W3, b3,
                        ln_g, ln_b, W6, b6, W8, b8)
    res = run_bass_kernel_spmd(nc, in_maps, list(range(cfg["n_cores"])))
    return assemble_output(cfg, res.results)
